# revision 13
# baseline (speedup 1.0000x reference)
"""Multi-head linear attention on Trainium2 — 8-core SPMD, batch+head sharded.

Full-tensor contract: kernel(**inputs) takes the complete Q/K/V
[4, 4096, 1024] f32 arrays, internally shards them across 8 NeuronCores
(core c -> batch c//2, heads 8*(c%2) .. 8*(c%2)+8, i.e. a contiguous
512-column slice of the embedding dim), runs one Bass kernel per core,
and reassembles the full [4, 4096, 1024] f32 output.

Per-core math (H=8 local heads, D=64, L=4096):
    phi = sigmoid(0.6053*x - 4.102)
    kv_ext[h] = phi_K[h]^T @ [V[h] | 1]     # [64, 65], f32 PSUM accum
    numden[h] = phi_Q[h] @ kv_ext[h]        # [L, 65]
    out[h]    = numden[h][:, :64] / numden[h][:, 64:65]

Layout: the host restacks each core's [4096, 512] slice to [8192, 256]
(head groups 0-3 / 4-7 stacked along rows) so the two 4-head groups
pipeline — group 0's phase-Q and division overlap group 1's K/V
streaming — while every DMA still moves 2 KiB-contiguous lines (each
SBUF partition line carries 2 consecutive L-rows; kv accumulation over
L is order-invariant, and the q-row permutation is undone because the
output uses the same 2-rows-per-partition layout the host unstacks).

Heads are processed in pairs: one K=128 matmul per pair computes both
heads' kv_ext blocks (phi_K pair chunk stationary, [V|1] pair moving;
off-diagonal blocks discarded), and one K=128 matmul per pair computes
both numden blocks against a block-diagonal kv operand. Q is
transposed raw on the PE (f32), sigmoid fuses the PSUM->SBUF copy on
ScalarE, V is cast f32->bf16 in-flight by SWDGE DMA. The division runs
on VectorE: per-chunk PSUM->SBUF copy, one batched reciprocal, one
broadcast multiply per row-tile. Matmul inputs are bf16 (PSUM
accumulation stays f32).
"""

import numpy as np

B = 4
L = 4096
E = 1024
NH = 8            # heads per core
D = 64
W = D + 1         # head block width incl. ones/den column
EC = NH * D       # 512 embedding columns per core
P = 128
G = 2             # head groups (4 heads each), stacked along rows
GC = EC // G      # 256 columns per group
NPAIR = GC // P   # head pairs per group (2)
SUB = 2           # L-rows per partition line (512 f32 = 2 KiB)
RT = SUB * GC     # 512 elements per partition line
NT = L // (P * SUB)   # 16 super-tiles (256 L-rows) per group
TBS = 4           # super-tiles per DMA batch -> 1 MiB loads
NBS = NT // TBS   # 4 batches per tensor per group
N_CORES = 8

_CACHE = {}


def _build_nc():
    from contextlib import ExitStack

    import concourse.bacc as bacc
    import concourse.bass as bass
    import concourse.mybir as mybir
    import concourse.tile as tile
    from concourse.masks import make_identity

    f32 = mybir.dt.float32
    bf16 = mybir.dt.bfloat16
    SIG = mybir.ActivationFunctionType.Sigmoid

    nc = bacc.Bacc("TRN2", target_bir_lowering=False, debug=False)
    Q = nc.dram_tensor("Q", [G * L, GC], f32, kind="ExternalInput").ap()
    K = nc.dram_tensor("K", [G * L, GC], f32, kind="ExternalInput").ap()
    V = nc.dram_tensor("V", [G * L, GC], f32, kind="ExternalInput").ap()
    O = nc.dram_tensor("O", [G * L, GC], f32, kind="ExternalOutput").ap()

    with tile.TileContext(nc) as tc, ExitStack() as ctx:
        singles = ctx.enter_context(tc.tile_pool(name="singles", bufs=1))
        ld = ctx.enter_context(tc.tile_pool(name="ld", bufs=3))
        vb = ctx.enter_context(tc.tile_pool(name="vb", bufs=3))
        ph = ctx.enter_context(tc.tile_pool(name="ph", bufs=3))
        qt = ctx.enter_context(tc.tile_pool(name="qt", bufs=64))
        rcp = ctx.enter_context(tc.tile_pool(name="rcp", bufs=4))
        sg = ctx.enter_context(tc.tile_pool(name="sg", bufs=2))
        ob = ctx.enter_context(tc.tile_pool(name="ob", bufs=2))
        pt = ctx.enter_context(tc.tile_pool(name="pt", bufs=2, space="PSUM"))
        pn = ctx.enter_context(tc.tile_pool(name="pn", bufs=2, space="PSUM"))
        pk = ctx.enter_context(tc.tile_pool(name="pk", bufs=1, space="PSUM"))

        ident = singles.tile([P, P], f32)
        make_identity(nc, ident)

        sig_bias = singles.tile([P, 1], f32)
        nc.vector.memset(sig_bias, -4.102)

        # Block-diagonal kv operand per head pair: rows 0:64 cols 0:65 hold
        # kv_ext of the even head, rows 64:128 cols 65:130 the odd head.
        kv_bd = singles.tile([P, G * NPAIR, 2 * W], bf16)
        nc.vector.memset(kv_bd, 0.0)

        kv_ps = [pk.tile([P, GC + 2], f32, tag=f"kv{i}", name=f"kv{i}")
                 for i in range(G * NPAIR)]

        for g in range(G):
            rbase = g * L

            # ---- K/V streaming: kv_pair += phiK_pair^T @ [V|1]_pair ----
            for ib in range(NBS):
                rows = slice(rbase + ib * TBS * P * SUB,
                             rbase + (ib + 1) * TBS * P * SUB)
                k_raw = ld.tile([P, TBS, RT], f32, tag="kraw", name="k_raw")
                nc.sync.dma_start(
                    out=k_raw,
                    in_=K[rows, :].rearrange("(t p s) e -> p t (s e)", p=P, s=SUB),
                )
                phiK = ph.tile([P, TBS, RT], bf16, tag="phiK", name="phiK")
                nc.scalar.activation(
                    out=phiK, in_=k_raw, func=SIG, bias=sig_bias, scale=0.6053
                )
                # [V_group(256) | 1 | 1] lines per (t, s): 512 B contiguous
                # DMA writes (no sub-512B read-modify-write), ones at the
                # tail so one matmul also accumulates k_sum in column 256.
                # Full-tile memset first: supplies the ones and forces the
                # scheduler to order memset -> DMA (overlapping regions).
                v_bf = vb.tile([P, TBS, SUB, GC + 2], bf16, name="v_bf")
                nc.gpsimd.memset(
                    v_bf.rearrange("p t s w -> p (t s) w"), 1.0)
                for t in range(TBS):
                    trows = slice(rbase + (ib * TBS + t) * P * SUB,
                                  rbase + (ib * TBS + t + 1) * P * SUB)
                    nc.gpsimd.dma_start(
                        out=v_bf[:, t, :, 0:GC],
                        in_=V[trows, :].rearrange("(p s) e -> p (s e)", s=SUB),
                    )
                for t in range(TBS):
                    for s in range(SUB):
                        for c in range(NPAIR):
                            nc.tensor.matmul(
                                out=kv_ps[g * NPAIR + c],
                                lhsT=phiK[:, t, s * GC + c * P:
                                          s * GC + (c + 1) * P],
                                rhs=v_bf[:, t, s, :],
                                start=(ib == 0 and t == 0 and s == 0),
                                stop=(ib == NBS - 1 and t == TBS - 1
                                      and s == SUB - 1),
                            )
            for c in range(NPAIR):
                pg = g * NPAIR + c
                nc.vector.tensor_copy(
                    out=kv_bd[0:D, pg, 0:D],
                    in_=kv_ps[pg][0:D, 2 * c * D:(2 * c + 1) * D])
                nc.vector.tensor_copy(
                    out=kv_bd[0:D, pg, D:W],
                    in_=kv_ps[pg][0:D, GC:GC + 1])
                nc.vector.tensor_copy(
                    out=kv_bd[D:P, pg, W:W + D],
                    in_=kv_ps[pg][D:P, (2 * c + 1) * D:(2 * c + 2) * D])
                nc.vector.tensor_copy(
                    out=kv_bd[D:P, pg, W + D:2 * W],
                    in_=kv_ps[pg][D:P, GC:GC + 1])

            # ---- Q streaming: transpose raw Q on PE, sigmoid PSUM->SBUF
            # on ACT, one matmul per pair, divide on DVE ----
            for ib in range(NBS):
                rows = slice(rbase + ib * TBS * P * SUB,
                             rbase + (ib + 1) * TBS * P * SUB)
                q_raw = ld.tile([P, TBS, RT], f32, tag="qraw", name="q_raw")
                nc.sync.dma_start(
                    out=q_raw,
                    in_=Q[rows, :].rearrange("(t p s) e -> p t (s e)", p=P, s=SUB),
                )
                # numden staging for the whole batch: 16 chunks of [2, 65]
                stg = sg.tile([P, TBS, SUB, NPAIR, 2, W], f32, name="stg")
                out_t = ob.tile([P, TBS, RT], f32, name="out_t")
                for t in range(TBS):
                    for s in range(SUB):
                        for c in range(NPAIR):
                            tp = pt.tile([P, P], f32, tag="tp", name="tp")
                            nc.tensor.transpose(
                                out=tp,
                                in_=q_raw[:, t, s * GC + c * P:
                                          s * GC + (c + 1) * P],
                                identity=ident,
                            )
                            qtT = qt.tile([P, P], bf16, tag="qtT", name="qtT")
                            nc.scalar.activation(
                                out=qtT, in_=tp, func=SIG, bias=sig_bias,
                                scale=0.6053,
                            )
                            num = pn.tile([P, 2, W], f32, tag="num", name="num")
                            nc.tensor.matmul(
                                out=num.rearrange("p a b -> p (a b)"),
                                lhsT=qtT,
                                rhs=kv_bd[:, g * NPAIR + c, :],
                            )
                            nc.vector.tensor_copy(
                                out=stg[:, t, s, c], in_=num)
                # one reciprocal for all 32 dens of the batch
                r = rcp.tile([P, TBS, SUB, NPAIR, 2], f32, name="r")
                nc.vector.reciprocal(
                    out=r,
                    in_=stg.rearrange("p t s c a w -> p (t s c a) w")[:, :, D],
                )
                # one broadcast multiply per super-tile: [p, 8 blocks, 64]
                for t in range(TBS):
                    r_bc = bass.AP(
                        tensor=r.tensor,
                        offset=r.offset + t * SUB * NPAIR * 2,
                        ap=[r.ap[0], [1, SUB * NPAIR * 2], [0, D]],
                    )
                    nc.vector.tensor_tensor(
                        out=out_t[:, t].rearrange("p (b d) -> p b d", d=D),
                        in0=stg[:, t].rearrange(
                            "p s c a w -> p (s c a) w")[:, :, 0:D],
                        in1=r_bc,
                        op=mybir.AluOpType.mult,
                    )
                nc.scalar.dma_start(
                    out=O[rows, :].rearrange("(t p s) e -> p t (s e)", p=P, s=SUB),
                    in_=out_t,
                )

    nc.compile()
    return nc


def _get_nc():
    if "nc" not in _CACHE:
        _CACHE["nc"] = _build_nc()
    return _CACHE["nc"]


def _shard(arr):
    """Full [B, L, E] f32 -> list of 8 per-core [2L, 256] group-stacked."""
    out = []
    for c in range(N_CORES):
        b, g = divmod(c, 2)
        sl = arr[b, :, g * EC:(g + 1) * EC]
        out.append(np.ascontiguousarray(
            np.concatenate([sl[:, 0:GC], sl[:, GC:EC]], axis=0)))
    return out


def run_sharded(in_maps, trace=False, trace_cores=None):
    from concourse.bass_utils import run_bass_kernel_spmd

    nc = _get_nc()
    kwargs = {}
    if trace:
        kwargs = dict(trace=True, trace_cores=trace_cores or [0])
    return run_bass_kernel_spmd(nc, in_maps, core_ids=list(range(N_CORES)), **kwargs)


def kernel(**inputs):
    Q = np.asarray(inputs["Q"], dtype=np.float32)
    K = np.asarray(inputs["K"], dtype=np.float32)
    V = np.asarray(inputs["V"], dtype=np.float32)
    qs, ks, vs = _shard(Q), _shard(K), _shard(V)
    in_maps = [{"Q": qs[c], "K": ks[c], "V": vs[c]} for c in range(N_CORES)]
    res = run_sharded(in_maps)
    out = np.empty((B, L, E), dtype=np.float32)
    for c in range(N_CORES):
        b, g = divmod(c, 2)
        o2 = res.results[c]["O"]
        out[b, :, g * EC:g * EC + GC] = o2[0:L]
        out[b, :, g * EC + GC:(g + 1) * EC] = o2[L:2 * L]
    return out
